# revision 1
# baseline (speedup 1.0000x reference)
"""GroupedQueryAttention Trainium2 kernel.

B=2, S=2048, D_MODEL=2048, 32 query heads / 8 KV heads, d_k=64.
Sharding: 8 cores = 2 (batch) x 4 (head groups of 8 query heads / 2 KV heads).
Per core: Wq/Wk/Wv column shard, Wo row shard; host sums the 4 partial
outputs per batch (the "all-reduce" of the row-parallel output projection).

Per-core device schedule (everything float32r on the PE at full rate):
  phase 1: Q^T, K^T, V^T projections from host-transposed x^T; V^T is
           PE-transposed back to natural [token, dim] layout and augmented
           with a ones column (softmax denominator rides the ctx matmul).
  phase 2: per head-pair, per 512-query tile: scores^T = K_T.T @ Q_T with the
           two heads row-packed into PE strips (rows 0-63 / 64-127, via a
           partition-duplicated K^T), exp on ScalarE straight out of PSUM
           (scale=1/8 folded into the activation), ctx^T accumulated as
           V_aug.T @ expS^T (m=65: 64 ctx dims + denominator row),
           normalization folded into the PSUM eviction.
  phase 3: partial output projection ctx^T.T @ Wo_rows -> DMA out.
"""

import sys

sys.path.insert(0, "/opt/trn_rl_repo")

import numpy as np

import concourse.bass as bass
import concourse.tile as tile
from concourse import bacc, mybir
from concourse.bass_utils import run_bass_kernel_spmd
from concourse.masks import make_identity

F32 = mybir.dt.float32
F32R = mybir.dt.float32r
F16 = mybir.dt.float16

D = 2048          # d_model
S = 2048          # sequence length
HL = 8            # query heads per core
KVL = 2           # kv heads per core
DK = 64
QO = HL * DK      # 512 query outdims per core
KO = KVL * DK     # 128 kv outdims per core
NKT = 16          # d_model contraction tiles of 128
NTT = 16          # token tiles of 128
NQT = 4           # query tiles of 512
EG = 2            # key-tiles per exp group

_CACHE = {}


def _build_nc():
    nc = bacc.Bacc("TRN2", target_bir_lowering=False)

    xT_h = nc.dram_tensor("xT", [D, S], F16, kind="ExternalInput")
    wq_h = nc.dram_tensor("wq", [D, QO], F16, kind="ExternalInput")
    wk_h = nc.dram_tensor("wk", [D, KO], F16, kind="ExternalInput")
    wv_h = nc.dram_tensor("wv", [D, KO], F16, kind="ExternalInput")
    wo_h = nc.dram_tensor("wo", [QO, D], F16, kind="ExternalInput")
    bq_h = nc.dram_tensor("bq2", [128, 4], F32, kind="ExternalInput")
    bk_h = nc.dram_tensor("bk2", [128, 1], F32, kind="ExternalInput")
    bv_h = nc.dram_tensor("bv2", [128, 1], F32, kind="ExternalInput")
    out_h = nc.dram_tensor("out", [S, D], F32, kind="ExternalOutput")

    with tile.TileContext(nc) as tc:
        _emit(nc, tc, xT_h, wq_h, wk_h, wv_h, wo_h, bq_h, bk_h, bv_h, out_h)
    nc.compile()
    return nc


def _emit(nc, tc, xT_h, wq_h, wk_h, wv_h, wo_h, bq_h, bk_h, bv_h, out_h):
    from contextlib import ExitStack

    ctx = ExitStack()
    with ctx:
        consts = ctx.enter_context(tc.tile_pool(name="consts", bufs=1))
        projout = ctx.enter_context(tc.tile_pool(name="projout", bufs=1))
        mmps = ctx.enter_context(tc.tile_pool(name="mmps", bufs=3, space="PSUM"))
        accps = ctx.enter_context(tc.tile_pool(name="accps", bufs=1, space="PSUM"))

        ident = consts.tile([128, 128], F16)
        make_identity(nc, ident)

        # persistent projection outputs
        qt_sb = projout.tile([128, 4, S], F16)    # [dim-in-pair, pair, token]
        ktd_sb = projout.tile([128, KVL, S], F16)  # kv dims duplicated both halves
        vt_sb = projout.tile([128, S], F16)        # [kv dims (2x64), token]
        vaug_sb = projout.tile([128, NTT, KVL, 128], F16)  # [tok, tok-tile, kv, dim|ones]
        ctxT_sb = projout.tile([128, 4, S], F16)   # [dim-in-pair, pair, token]

        bq_sb = consts.tile([128, 4], F32)
        bk_sb = consts.tile([128, 1], F32)
        bv_sb = consts.tile([128, 1], F32)
        nc.sync.dma_start(out=bq_sb, in_=bq_h[:])
        nc.sync.dma_start(out=bk_sb, in_=bk_h[:])
        nc.sync.dma_start(out=bv_sb, in_=bv_h[:])

        # ---------------- phase 1: projections ----------------
        with tc.tile_pool(name="xt", bufs=4) as xtp, \
             tc.tile_pool(name="wqkv", bufs=1) as wp:
            wq_sb = wp.tile([128, NKT, QO], F16)
            wk_sb = wp.tile([128, NKT, KO], F16)
            wv_sb = wp.tile([128, NKT, KO], F16)
            nc.sync.dma_start(out=wq_sb, in_=wq_h.rearrange("(k p) m -> p k m", p=128))
            nc.sync.dma_start(out=wk_sb, in_=wk_h.rearrange("(k p) m -> p k m", p=128))
            nc.sync.dma_start(out=wv_sb, in_=wv_h.rearrange("(k p) m -> p k m", p=128))

            xT_r = xT_h.rearrange("(k p) t -> p k t", p=128)
            xts = []
            for nt in range(4):  # 512-token slabs, all held resident
                ns = slice(nt * 512, (nt + 1) * 512)
                xt_t = xtp.tile([128, NKT, 512], F16, tag="xt", name=f"xt{nt}")
                nc.sync.dma_start(out=xt_t, in_=xT_r[:, :, ns])
                xts.append(xt_t)

            # K then V first (attention needs them for every query tile),
            # Q last so attention can start while late Q slabs project.
            for nt in range(4):
                ns = slice(nt * 512, (nt + 1) * 512)
                ps = mmps.tile([128, EG, 512], F32)
                for kt in range(NKT):
                    nc.tensor.matmul(
                        ps[:, 0, :], lhsT=wk_sb[:, kt, :], rhs=xts[nt][:, kt, :],
                        start=(kt == 0), stop=(kt == NKT - 1))
                nc.vector.tensor_scalar_add(ps[:, 1, :], ps[:, 0, :],
                                            bk_sb[:, 0:1])
                for kv in range(KVL):
                    src = ps[kv * 64:(kv + 1) * 64, 1, 0:512]
                    nc.vector.tensor_copy(ktd_sb[0:64, kv, ns], src)
                    nc.vector.tensor_copy(ktd_sb[64:128, kv, ns], src)

            for nt in range(4):
                ns = slice(nt * 512, (nt + 1) * 512)
                ps = mmps.tile([128, EG, 512], F32)
                for kt in range(NKT):
                    nc.tensor.matmul(
                        ps[:, 0, :], lhsT=wv_sb[:, kt, :], rhs=xts[nt][:, kt, :],
                        start=(kt == 0), stop=(kt == NKT - 1))
                nc.vector.tensor_scalar_add(vt_sb[:, ns], ps[:, 0, :], bv_sb[:, 0:1])

            for nt in range(4):
                ns = slice(nt * 512, (nt + 1) * 512)
                for mt in range(4):  # Q^T m-tiles (= head pairs)
                    ps = mmps.tile([128, EG, 512], F32)
                    for kt in range(NKT):
                        nc.tensor.matmul(
                            ps[:, 0, :],
                            lhsT=wq_sb[:, kt, mt * 128:(mt + 1) * 128],
                            rhs=xts[nt][:, kt, :],
                            start=(kt == 0), stop=(kt == NKT - 1),
                        )
                    nc.vector.tensor_scalar_add(
                        qt_sb[:, mt, ns], ps[:, 0, :], bq_sb[:, mt:mt + 1])

        # V^T -> natural V layout via PE transpose, build V_aug with ones col
        for tt in range(NTT):
            pst = mmps.tile([128, EG, 512], F16, tag="ps", name="pst")
            nc.tensor.transpose(
                pst[:, 0, 0:128],
                vt_sb[:, tt * 128:(tt + 1) * 128],
                ident[:],
            )
            for kv in range(KVL):
                nc.vector.tensor_copy(
                    vaug_sb[:, tt, kv, 0:64], pst[:, 0, kv * 64:(kv + 1) * 64])
        ones_sb = consts.tile([128, 64], F16)
        nc.vector.memset(ones_sb, 1.0)
        for tt in range(NTT):
            for kv in range(KVL):
                nc.vector.tensor_copy(vaug_sb[:, tt, kv, 64:128], ones_sb)

        # ---------------- phase 2: attention ----------------
        with tc.tile_pool(name="expst", bufs=6) as ep, \
             tc.tile_pool(name="rden", bufs=4) as rp, \
             tc.tile_pool(name="wo", bufs=1) as wop:
            wo_sb = wop.tile([128, 4, D], F16)
            nc.sync.dma_start(out=wo_sb, in_=wo_h.rearrange("(c p) d -> p c d", p=128))

            for qt in range(NQT):
                qs = slice(qt * 512, (qt + 1) * 512)
                for pair in range(4):
                    kv = pair // 2
                    ctx_ps = [accps.tile([128, 512], F32, tag=f"ctx{i}", name=f"ctx{i}") for i in range(2)]
                    for g in range(NTT // EG):
                        sp = [mmps.tile([128, EG, 512], F32, tag="ps", name=f"sp{i}") for i in range(2)]
                        for j in range(EG):
                            ktile = g * EG + j
                            ks = slice(ktile * 128, (ktile + 1) * 128)
                            for i in range(2):  # head i of the pair
                                nc.tensor.matmul(
                                    sp[i][:, j, :],
                                    lhsT=ktd_sb[i * 64:(i + 1) * 64, kv, ks],
                                    rhs=qt_sb[i * 64:(i + 1) * 64, pair, qs],
                                    start=True, stop=True,
                                    tile_position=(i * 64, 0),
                                )
                        es = []
                        for i in range(2):
                            e = ep.tile([128, EG, 512], F16)
                            nc.scalar.activation(
                                e[:, :, :], sp[i][:, :, :],
                                mybir.ActivationFunctionType.Exp, scale=0.125)
                            es.append(e)
                        for j in range(EG):
                            ktile = g * EG + j
                            for i in range(2):
                                nc.tensor.matmul(
                                    ctx_ps[i][:, :],
                                    lhsT=vaug_sb[:, ktile, kv, :],
                                    rhs=es[i][:, j, :],
                                    start=(ktile == 0), stop=(ktile == NTT - 1),
                                )
                    for i in range(2):
                        rdb = rp.tile([64, 512], F32)
                        nc.vector.reciprocal(rdb, ctx_ps[i][64:128, :])
                        nc.vector.tensor_tensor(
                            ctxT_sb[i * 64:(i + 1) * 64, pair, qs],
                            ctx_ps[i][0:64, :],
                            rdb[:, :],
                            mybir.AluOpType.mult,
                        )

            # ---------------- phase 3: output projection ----------------
            with tc.tile_pool(name="osb", bufs=4) as op:
                for tt in range(NTT):
                    ts_ = slice(tt * 128, (tt + 1) * 128)
                    for dn in range(4):
                        ds_ = slice(dn * 512, (dn + 1) * 512)
                        ps = mmps.tile([128, EG, 512], F32)
                        for c in range(4):
                            nc.tensor.matmul(
                                ps[:, 0, :],
                                lhsT=ctxT_sb[:, c, ts_],
                                rhs=wo_sb[:, c, ds_],
                                start=(c == 0), stop=(c == 3),
                            )
                        ob = op.tile([128, 512], F32)
                        nc.vector.tensor_copy(ob, ps[:, 0, :])
                        nc.sync.dma_start(out=out_h[ts_, ds_], in_=ob)


def _get_nc():
    if "nc" not in _CACHE:
        _CACHE["nc"] = _build_nc()
    return _CACHE["nc"]


def kernel(x, Wq, bq, Wk, bk, Wv, bv, Wo, bo, _trace=False):
    x = np.asarray(x, np.float32)
    Wq = np.asarray(Wq, np.float32)
    bq = np.asarray(bq, np.float32)
    Wk = np.asarray(Wk, np.float32)
    bk = np.asarray(bk, np.float32)
    Wv = np.asarray(Wv, np.float32)
    bv = np.asarray(bv, np.float32)
    Wo = np.asarray(Wo, np.float32)
    bo = np.asarray(bo, np.float32)

    nc = _get_nc()
    in_maps = []
    for r in range(8):
        b, g = divmod(r, 4)
        qsl = slice(g * 512, (g + 1) * 512)
        ksl = slice(g * 128, (g + 1) * 128)
        in_maps.append({
            "xT": np.ascontiguousarray(x[b].T.astype(np.float16)),
            "wq": np.ascontiguousarray(Wq[:, qsl].astype(np.float16)),
            "wk": np.ascontiguousarray(Wk[:, ksl].astype(np.float16)),
            "wv": np.ascontiguousarray(Wv[:, ksl].astype(np.float16)),
            "wo": np.ascontiguousarray(Wo[qsl, :].astype(np.float16)),
            "bq2": np.ascontiguousarray(bq[qsl].reshape(4, 128).T),
            "bk2": np.ascontiguousarray(bk[ksl].reshape(128, 1)),
            "bv2": np.ascontiguousarray(bv[ksl].reshape(128, 1)),
        })

    res = run_bass_kernel_spmd(nc, in_maps, list(range(8)), trace=_trace)
    out = np.zeros((2, S, D), np.float64)
    for r in range(8):
        out[r // 4] += res.results[r]["out"].astype(np.float64)
    out += bo.astype(np.float64)
    result = out.astype(np.float32)
    if _trace:
        return result, res
    return result



# revision 9
# speedup vs baseline: 1.3036x; 1.3036x over previous
"""GroupedQueryAttention Trainium2 kernel.

B=2, S=2048, D_MODEL=2048, 32 query heads / 8 KV heads, d_k=64.
Sharding: 8 cores = 2 (batch) x 4 (head groups of 8 query heads / 2 KV heads).
Per core: Wq/Wk/Wv column shard, Wo row shard; host sums the 4 partial
outputs per batch (the "all-reduce" of the row-parallel output projection).

Per-core schedule (v2 - PE-saturating interleave):
  The PE's ~358us of matmul work is the kernel floor; ScalarE exp (~285us)
  and everything else must hide underneath it. Idle PE gaps also re-throttle
  the HAM clock gate (PE drops 2.4->1.2GHz), so the emission order keeps the
  PE stream dense:
    prologue: K proj, V proj, V transpose->V_aug, Q proj slab 0
    main loop over (query-tile, head-pair): scores -> exp (ScalarE) -> ctx,
      with a background queue of PE work (remaining Q-proj slabs, output
      projection of finished query tiles) drained between score/ctx groups
      to fill the exp-latency bubbles.
  Normalization uses reciprocal_approx_fast (single-pass custom DVE op)
  instead of the 8-cycle/element iterative reciprocal. PSUM->SBUF evictions
  of the output projection run on the otherwise-idle GpSimd engine. Output
  is written f16 (host accumulates partials in f64).

PSUM (8 banks): scores pool 3x[128,2,512]f32 = 6 banks (also serves the
projection / output-projection accumulators), ctx accumulators 2x1 bank.
"""

import sys

sys.path.insert(0, "/opt/trn_rl_repo")

from collections import deque

import numpy as np

import concourse.bass as bass
import concourse.tile as tile
from concourse import bacc, mybir
from concourse.bass_utils import run_bass_kernel_spmd
from concourse.masks import make_identity

F32 = mybir.dt.float32
F16 = mybir.dt.float16

D = 2048          # d_model
S = 2048          # sequence length
HL = 8            # query heads per core
KVL = 2           # kv heads per core
DK = 64
QO = HL * DK      # 512 query outdims per core
KO = KVL * DK     # 128 kv outdims per core
NKT = 16          # d_model contraction tiles of 128
NTT = 16          # token tiles of 128
NQT = 4           # query tiles of 512
EG = 2            # key-tiles per exp group

_CACHE = {}


def _build_nc():
    nc = bacc.Bacc("TRN2", target_bir_lowering=False)

    xT_h = nc.dram_tensor("xT", [D, S], F16, kind="ExternalInput")
    wq_h = nc.dram_tensor("wq", [D, QO], F16, kind="ExternalInput")
    wk_h = nc.dram_tensor("wk", [D, KO], F16, kind="ExternalInput")
    wv_h = nc.dram_tensor("wv", [D, KO], F16, kind="ExternalInput")
    wo_h = nc.dram_tensor("wo", [QO, D], F16, kind="ExternalInput")
    bq_h = nc.dram_tensor("bq2", [128, 4], F32, kind="ExternalInput")
    bk_h = nc.dram_tensor("bk2", [128, 1], F32, kind="ExternalInput")
    bv_h = nc.dram_tensor("bv2", [128, 1], F32, kind="ExternalInput")
    out_h = nc.dram_tensor("out", [S, D], F16, kind="ExternalOutput")

    with tile.TileContext(nc) as tc:
        _emit(nc, tc, xT_h, wq_h, wk_h, wv_h, wo_h, bq_h, bk_h, bv_h, out_h)
    nc.compile()
    return nc


def _emit(nc, tc, xT_h, wq_h, wk_h, wv_h, wo_h, bq_h, bk_h, bv_h, out_h):
    from contextlib import ExitStack

    ctx = ExitStack()
    with ctx:
        consts = ctx.enter_context(tc.tile_pool(name="consts", bufs=1))
        projout = ctx.enter_context(tc.tile_pool(name="projout", bufs=1))
        scps = ctx.enter_context(tc.tile_pool(name="scps", bufs=3, space="PSUM"))
        cxps = ctx.enter_context(tc.tile_pool(name="cxps", bufs=2, space="PSUM"))
        ep = ctx.enter_context(tc.tile_pool(name="expst", bufs=6))
        rp = ctx.enter_context(tc.tile_pool(name="rden", bufs=4))
        xtp = ctx.enter_context(tc.tile_pool(name="xt", bufs=4))
        wp = ctx.enter_context(tc.tile_pool(name="wqkv", bufs=1))
        op = ctx.enter_context(tc.tile_pool(name="osb", bufs=4))

        ident = consts.tile([128, 128], F16)
        make_identity(nc, ident)

        # persistent projection outputs
        qt_sb = projout.tile([128, 4, S], F16)     # [dim-in-pair, pair, token]
        ktd_sb = projout.tile([128, KVL, S], F16)  # kv dims duplicated both halves
        vt_sb = projout.tile([128, S], F16)        # [kv dims (2x64), token]
        vaug_sb = projout.tile([128, NTT, KVL, 128], F16)  # [tok, tok-tile, kv, dim|ones]
        ctxT_sb = projout.tile([128, 4, S], F16)   # [dim-in-pair, pair, token]

        bq_sb = consts.tile([128, 4], F32)
        bk_sb = consts.tile([128, 1], F32)
        bv_sb = consts.tile([128, 1], F32)
        nc.sync.dma_start(out=bk_sb, in_=bk_h[:])
        nc.sync.dma_start(out=bv_sb, in_=bv_h[:])
        nc.sync.dma_start(out=bq_sb, in_=bq_h[:])

        # ---------------- input DMAs (K/V weights + x slabs first) ---------
        wq_sb = wp.tile([128, NKT, QO], F16)
        wk_sb = wp.tile([128, NKT, KO], F16)
        wv_sb = wp.tile([128, NKT, KO], F16)
        wo_sb = wp.tile([128, 4, D], F16)
        nc.sync.dma_start(out=wk_sb, in_=wk_h.rearrange("(k p) m -> p k m", p=128))
        nc.sync.dma_start(out=wv_sb, in_=wv_h.rearrange("(k p) m -> p k m", p=128))

        xT_r = xT_h.rearrange("(k p) t -> p k t", p=128)
        xts = []
        for nt in range(4):  # 512-token slabs, all held resident
            ns = slice(nt * 512, (nt + 1) * 512)
            xt_t = xtp.tile([128, NKT, 512], F16, tag="xt", name=f"xt{nt}")
            nc.sync.dma_start(out=xt_t, in_=xT_r[:, :, ns])
            xts.append(xt_t)
        nc.sync.dma_start(out=wq_sb, in_=wq_h.rearrange("(k p) m -> p k m", p=128))
        nc.sync.dma_start(out=wo_sb, in_=wo_h.rearrange("(c p) d -> p c d", p=128))

        # Fill V_aug with ones once; V-dim columns are overwritten below.
        # (Same engine as the column copies -> strictly ordered.)
        nc.vector.memset(vaug_sb, 1.0)

        def sc_tile(name):
            return scps.tile([128, EG, 512], F32, tag="sc", name=name)

        # ---------------- K projection (all slabs) -------------------------
        for nt in range(4):
            ns = slice(nt * 512, (nt + 1) * 512)
            ps = sc_tile(f"kp{nt}")
            for kt in range(NKT):
                nc.tensor.matmul(ps[:, 0, :], lhsT=wk_sb[:, kt, :],
                                 rhs=xts[nt][:, kt, :],
                                 start=(kt == 0), stop=(kt == NKT - 1))
            for kv in range(KVL):
                nc.vector.tensor_scalar_add(
                    ktd_sb[0:64, kv, ns], ps[kv * 64:(kv + 1) * 64, 0, :],
                    bk_sb[kv * 64:kv * 64 + 64, 0:1])
                nc.vector.tensor_copy(ktd_sb[64:128, kv, ns], ktd_sb[0:64, kv, ns])

        # ---------------- V projection (all slabs) -------------------------
        for nt in range(4):
            ns = slice(nt * 512, (nt + 1) * 512)
            ps = sc_tile(f"vp{nt}")
            for kt in range(NKT):
                nc.tensor.matmul(ps[:, 0, :], lhsT=wv_sb[:, kt, :],
                                 rhs=xts[nt][:, kt, :],
                                 start=(kt == 0), stop=(kt == NKT - 1))
            nc.vector.tensor_scalar_add(vt_sb[:, ns], ps[:, 0, :], bv_sb[:, 0:1])

        # V^T -> natural V layout via PE transpose into V_aug dim columns
        for tt in range(NTT):
            pst = scps.tile([128, EG, 512], F16, tag="sc", name=f"tp{tt}")
            nc.tensor.transpose(
                pst[:, 0, 0:128], vt_sb[:, tt * 128:(tt + 1) * 128], ident[:])
            for kv in range(KVL):
                nc.vector.tensor_copy(
                    vaug_sb[:, tt, kv, 0:64], pst[:, 0, kv * 64:(kv + 1) * 64])

        # ---------------- Q projection helper -------------------------------
        def q_proj_tile(nt, mt):
            ns = slice(nt * 512, (nt + 1) * 512)
            ps = sc_tile(f"q{nt}{mt}")
            for kt in range(NKT):
                nc.tensor.matmul(
                    ps[:, 0, :], lhsT=wq_sb[:, kt, mt * 128:(mt + 1) * 128],
                    rhs=xts[nt][:, kt, :],
                    start=(kt == 0), stop=(kt == NKT - 1))
            nc.vector.tensor_scalar_add(
                qt_sb[:, mt, ns], ps[:, 0, :], bq_sb[:, mt:mt + 1])

        # Q slab 0 upfront (attention on qt=0 needs it)
        for mt in range(4):
            q_proj_tile(0, mt)

        # ---------------- output projection helper ---------------------------
        def outproj_tile(tt, dn, ob_box):
            ts_ = slice(tt * 128, (tt + 1) * 128)
            ds_ = slice(dn * 512, (dn + 1) * 512)
            if dn == 0:
                ob_box[0] = op.tile([128, D], F16, tag="ob", name=f"ob{tt}")
            ps = sc_tile(f"op{tt}{dn}")
            for c in range(4):
                nc.tensor.matmul(
                    ps[:, 0, :], lhsT=ctxT_sb[:, c, ts_], rhs=wo_sb[:, c, ds_],
                    start=(c == 0), stop=(c == 3))
            nc.vector.tensor_copy(ob_box[0][:, ds_], ps[:, 0, :])
            if dn == 3:
                nc.sync.dma_start(out=out_h[ts_, :], in_=ob_box[0])

        # ---------------- background PE work queue ---------------------------
        # Items: (cost_ns, label, emit_fn). Drained between attention groups
        # to keep the PE busy during exp-latency bubbles. PE executes in
        # program order, so any producer a later instruction waits on must
        # already be emitted - hence the forced per-slab drain below.
        bg_queue = deque()
        budget = [0.0]

        def bg_tick(credit):
            budget[0] += credit
            while bg_queue and budget[0] >= bg_queue[0][0]:
                cost, _, fn = bg_queue.popleft()
                fn()
                budget[0] -= cost

        def bg_drain_label(lbl):
            while any(item[1] == lbl for item in bg_queue):
                _, _, fn = bg_queue.popleft()
                fn()
            budget[0] = 0.0

        def bg_drain_all():
            while bg_queue:
                _, _, fn = bg_queue.popleft()
                fn()

        for nt in range(1, 4):
            for mt in range(4):
                bg_queue.append(
                    (3600.0, f"q{nt}",
                     (lambda nt=nt, mt=mt: q_proj_tile(nt, mt))))

        def queue_outproj(qt):
            for tt in range(qt * 4, qt * 4 + 4):
                ob_box = [None]
                for dn in range(4):
                    bg_queue.append(
                        (900.0, f"o{qt}",
                         (lambda tt=tt, dn=dn, ob_box=ob_box:
                          outproj_tile(tt, dn, ob_box))))

        # ---------------- attention main loop -------------------------------
        for qt in range(NQT):
            if qt > 0:
                bg_drain_label(f"q{qt}")
            qs = slice(qt * 512, (qt + 1) * 512)
            for pair in range(4):
                kv = pair // 2
                ctx_ps = [cxps.tile([128, 512], F32, tag="cx", name=f"ctx{i}")
                          for i in range(2)]
                for g in range(NTT // EG):
                    sp = [sc_tile(f"sp{i}") for i in range(2)]
                    for j in range(EG):
                        ktile = g * EG + j
                        ks = slice(ktile * 128, (ktile + 1) * 128)
                        for i in range(2):  # head i of the pair
                            nc.tensor.matmul(
                                sp[i][:, j, :],
                                lhsT=ktd_sb[i * 64:(i + 1) * 64, kv, ks],
                                rhs=qt_sb[i * 64:(i + 1) * 64, pair, qs],
                                start=True, stop=True,
                                tile_position=(i * 64, 0),
                            )
                    es = []
                    for i in range(2):
                        e = ep.tile([128, EG, 512], F16)
                        nc.scalar.activation(
                            e[:, :, :], sp[i][:, :, :],
                            mybir.ActivationFunctionType.Exp, scale=0.125)
                        es.append(e)
                    for j in range(EG):
                        ktile = g * EG + j
                        for i in range(2):
                            nc.tensor.matmul(
                                ctx_ps[i][:, :],
                                lhsT=vaug_sb[:, ktile, kv, :],
                                rhs=es[i][:, j, :],
                                start=(ktile == 0), stop=(ktile == NTT - 1),
                            )
                    bg_tick(700.0)
                for i in range(2):
                    # custom-DVE ops don't honor partition offsets on HW:
                    # stage the denominator rows into an aligned tile first.
                    den = rp.tile([64, 512], F32, tag="den")
                    rdb = rp.tile([64, 512], F32, tag="rdb")
                    nc.vector.tensor_copy(den, ctx_ps[i][64:128, :])
                    nc.vector.reciprocal_approx_fast(rdb, den)
                    nc.vector.tensor_tensor(
                        ctxT_sb[i * 64:(i + 1) * 64, pair, qs],
                        ctx_ps[i][0:64, :],
                        rdb[:, :],
                        mybir.AluOpType.mult,
                    )
                bg_tick(1500.0)
            queue_outproj(qt)

        # tail: drain all remaining background work (late outproj tiles)
        bg_drain_all()


def _get_nc():
    if "nc" not in _CACHE:
        _CACHE["nc"] = _build_nc()
    return _CACHE["nc"]


def kernel(x, Wq, bq, Wk, bk, Wv, bv, Wo, bo, _trace=False):
    x = np.asarray(x, np.float32)
    Wq = np.asarray(Wq, np.float32)
    bq = np.asarray(bq, np.float32)
    Wk = np.asarray(Wk, np.float32)
    bk = np.asarray(bk, np.float32)
    Wv = np.asarray(Wv, np.float32)
    bv = np.asarray(bv, np.float32)
    Wo = np.asarray(Wo, np.float32)
    bo = np.asarray(bo, np.float32)

    nc = _get_nc()
    in_maps = []
    for r in range(8):
        b, g = divmod(r, 4)
        qsl = slice(g * 512, (g + 1) * 512)
        ksl = slice(g * 128, (g + 1) * 128)
        in_maps.append({
            "xT": np.ascontiguousarray(x[b].T.astype(np.float16)),
            "wq": np.ascontiguousarray(Wq[:, qsl].astype(np.float16)),
            "wk": np.ascontiguousarray(Wk[:, ksl].astype(np.float16)),
            "wv": np.ascontiguousarray(Wv[:, ksl].astype(np.float16)),
            "wo": np.ascontiguousarray(Wo[qsl, :].astype(np.float16)),
            "bq2": np.ascontiguousarray(bq[qsl].reshape(4, 128).T),
            "bk2": np.ascontiguousarray(bk[ksl].reshape(128, 1)),
            "bv2": np.ascontiguousarray(bv[ksl].reshape(128, 1)),
        })

    res = run_bass_kernel_spmd(nc, in_maps, list(range(8)), trace=_trace)
    out = np.zeros((2, S, D), np.float64)
    for r in range(8):
        out[r // 4] += res.results[r]["out"].astype(np.float64)
    out += bo.astype(np.float64)
    result = out.astype(np.float32)
    if _trace:
        return result, res
    return result


# revision 11
# speedup vs baseline: 1.3560x; 1.0403x over previous
"""GroupedQueryAttention Trainium2 kernel.

B=2, S=2048, D_MODEL=2048, 32 query heads / 8 KV heads, d_k=64.
Sharding: 8 cores = 2 (batch) x 4 (head groups of 8 query heads / 2 KV heads).
Per core: Wq/Wk/Wv column shard, Wo row shard; host sums the 4 partial
outputs per batch (the "all-reduce" of the row-parallel output projection).

Per-core schedule (v4 - PE-saturating interleave):
  The PE's ~360us of matmul work is the kernel floor; ScalarE exp (~270us)
  and everything else must hide underneath it. Idle PE gaps also re-throttle
  the HAM clock gate (PE drops 2.4->1.2GHz), so the emission order keeps the
  PE stream dense:
    prologue: K proj, V proj, V transpose->V_aug, Q proj slab 0
    main loop over (query-tile, head-pair): scores -> exp (ScalarE) -> ctx,
      with a background queue of PE work (remaining Q-proj slabs, output
      projection of finished query tiles) drained between score/ctx groups
      to fill the exp-latency bubbles.
  Heads are paired (h, h+4) so a pair's two heads use the core's two KV
  heads: the K-projection PSUM [kv0 dims | kv1 dims] is evicted once with
  no partition duplication, and scores pack both heads into the PE via
  tile_position row strips. The softmax denominator rides the ctx matmul
  as ones-columns of V_aug. Normalization uses reciprocal_approx_fast on a
  partition-aligned staging tile (custom-DVE ops ignore partition offsets
  on HW). Output is written f16; host accumulates partials in f64.

PSUM (8 banks): scores pool 3x[128,2,512]f32 = 6 banks (also serves the
projection / output-projection accumulators), ctx accumulators 2x1 bank.
"""

import sys

sys.path.insert(0, "/opt/trn_rl_repo")

from collections import deque

import numpy as np

import concourse.bass as bass
import concourse.tile as tile
from concourse import bacc, mybir
from concourse.bass_utils import run_bass_kernel_spmd
from concourse.masks import make_identity

F32 = mybir.dt.float32
F16 = mybir.dt.float16

D = 2048          # d_model
S = 2048          # sequence length
HL = 8            # query heads per core
KVL = 2           # kv heads per core
DK = 64
QO = HL * DK      # 512 query outdims per core
KO = KVL * DK     # 128 kv outdims per core
NKT = 16          # d_model contraction tiles of 128
NTT = 16          # token tiles of 128
NQT = 4           # query tiles of 512
EG = 2            # key-tiles per exp group

_CACHE = {}


def _build_nc():
    nc = bacc.Bacc("TRN2", target_bir_lowering=False)

    xT_h = nc.dram_tensor("xT", [D, S], F16, kind="ExternalInput")
    wq_h = nc.dram_tensor("wq", [D, QO], F16, kind="ExternalInput")
    wk_h = nc.dram_tensor("wk", [D, KO], F16, kind="ExternalInput")
    wv_h = nc.dram_tensor("wv", [D, KO], F16, kind="ExternalInput")
    wo_h = nc.dram_tensor("wo", [QO, D], F16, kind="ExternalInput")
    bq_h = nc.dram_tensor("bq2", [128, 4], F32, kind="ExternalInput")
    bk_h = nc.dram_tensor("bk2", [128, 1], F32, kind="ExternalInput")
    bv_h = nc.dram_tensor("bv2", [128, 1], F32, kind="ExternalInput")
    out_h = nc.dram_tensor("out", [S, D], F16, kind="ExternalOutput")

    with tile.TileContext(nc) as tc:
        _emit(nc, tc, xT_h, wq_h, wk_h, wv_h, wo_h, bq_h, bk_h, bv_h, out_h)
    nc.compile()
    return nc


def _emit(nc, tc, xT_h, wq_h, wk_h, wv_h, wo_h, bq_h, bk_h, bv_h, out_h):
    from contextlib import ExitStack

    ctx = ExitStack()
    with ctx:
        consts = ctx.enter_context(tc.tile_pool(name="consts", bufs=1))
        projout = ctx.enter_context(tc.tile_pool(name="projout", bufs=1))
        scps = ctx.enter_context(tc.tile_pool(name="scps", bufs=3, space="PSUM"))
        cxps = ctx.enter_context(tc.tile_pool(name="cxps", bufs=2, space="PSUM"))
        ep = ctx.enter_context(tc.tile_pool(name="expst", bufs=6))
        rp = ctx.enter_context(tc.tile_pool(name="rden", bufs=4))
        xtp = ctx.enter_context(tc.tile_pool(name="xt", bufs=8))
        wp = ctx.enter_context(tc.tile_pool(name="wqkv", bufs=1))
        op = ctx.enter_context(tc.tile_pool(name="osb", bufs=4))

        ident = consts.tile([128, 128], F16)
        make_identity(nc, ident)

        # persistent projection outputs
        qt_sb = projout.tile([128, 4, S], F16)     # [dim-in-pair, pair, token]
        kt_sb = projout.tile([128, S], F16)        # [kv0 dims | kv1 dims, token]
        vt_sb = projout.tile([128, S], F16)        # [kv0 dims | kv1 dims, token]
        vaug_sb = projout.tile([128, NTT, KVL, 128], F16)  # [tok, tok-tile, kv, dim|ones]
        ctxT_sb = projout.tile([128, 4, S], F16)   # [dim-in-pair, pair, token]

        bq_sb = consts.tile([128, 4], F32)
        bk_sb = consts.tile([128, 1], F32)
        bv_sb = consts.tile([128, 1], F32)
        nc.sync.dma_start(out=bk_sb, in_=bk_h[:])
        nc.sync.dma_start(out=bv_sb, in_=bv_h[:])
        nc.sync.dma_start(out=bq_sb, in_=bq_h[:])

        # ---------------- input DMAs (K weights + first x half-slab first) --
        wq_sb = wp.tile([128, NKT, QO], F16)
        wk_sb = wp.tile([128, NKT, KO], F16)
        wv_sb = wp.tile([128, NKT, KO], F16)
        wo_sb = wp.tile([128, 4, D], F16)
        nc.sync.dma_start(out=wk_sb, in_=wk_h.rearrange("(k p) m -> p k m", p=128))

        xT_r = xT_h.rearrange("(k p) t -> p k t", p=128)
        xts = []  # per 512-token slab: two half-tiles of 8 k-tiles each
        for nt in range(4):
            ns = slice(nt * 512, (nt + 1) * 512)
            halves = []
            for hf in range(2):
                xt_t = xtp.tile([128, NKT // 2, 512], F16, tag="xt",
                                name=f"xt{nt}h{hf}")
                nc.sync.dma_start(
                    out=xt_t, in_=xT_r[:, hf * 8:(hf + 1) * 8, ns])
                halves.append(xt_t)
            xts.append(halves)
            if nt == 0:
                nc.sync.dma_start(
                    out=wv_sb, in_=wv_h.rearrange("(k p) m -> p k m", p=128))
        nc.sync.dma_start(out=wq_sb, in_=wq_h.rearrange("(k p) m -> p k m", p=128))
        nc.sync.dma_start(out=wo_sb, in_=wo_h.rearrange("(c p) d -> p c d", p=128))

        def xslab(nt, kt):
            return xts[nt][kt // 8][:, kt % 8, :]

        # Ones-columns of V_aug (denominator rides the ctx matmul).
        # Same engine as the V-column copies below -> strictly ordered.
        nc.vector.memset(vaug_sb[:, :, :, 64:128], 1.0)

        def sc_tile(name):
            return scps.tile([128, EG, 512], F32, tag="sc", name=name)

        # ---------------- K projection (all slabs) -------------------------
        # PSUM rows = [kv0 d0-63 | kv1 d0-63]; evicted as-is (head pairing
        # (h, h+4) means score strips want exactly this layout).
        for nt in range(4):
            ns = slice(nt * 512, (nt + 1) * 512)
            ps = sc_tile(f"kp{nt}")
            for kt in range(NKT):
                nc.tensor.matmul(ps[:, 0, :], lhsT=wk_sb[:, kt, :],
                                 rhs=xslab(nt, kt),
                                 start=(kt == 0), stop=(kt == NKT - 1))
            nc.vector.tensor_scalar_add(kt_sb[:, ns], ps[:, 0, :], bk_sb[:, 0:1])

        # ---------------- V projection (all slabs) -------------------------
        for nt in range(4):
            ns = slice(nt * 512, (nt + 1) * 512)
            ps = sc_tile(f"vp{nt}")
            for kt in range(NKT):
                nc.tensor.matmul(ps[:, 0, :], lhsT=wv_sb[:, kt, :],
                                 rhs=xslab(nt, kt),
                                 start=(kt == 0), stop=(kt == NKT - 1))
            nc.vector.tensor_scalar_add(vt_sb[:, ns], ps[:, 0, :], bv_sb[:, 0:1])

        # V^T -> natural V layout via PE transpose into V_aug dim columns
        for tt in range(NTT):
            pst = scps.tile([128, EG, 512], F16, tag="sc", name=f"tp{tt}")
            nc.tensor.transpose(
                pst[:, 0, 0:128], vt_sb[:, tt * 128:(tt + 1) * 128], ident[:])
            nc.vector.tensor_copy(
                vaug_sb[:, tt, :, 0:64], pst[:, 0, 0:128])

        # ---------------- Q projection helper -------------------------------
        def q_proj_tile(nt, mt):
            ns = slice(nt * 512, (nt + 1) * 512)
            ps = sc_tile(f"q{nt}{mt}")
            for kt in range(NKT):
                nc.tensor.matmul(
                    ps[:, 0, :], lhsT=wq_sb[:, kt, mt * 128:(mt + 1) * 128],
                    rhs=xslab(nt, kt),
                    start=(kt == 0), stop=(kt == NKT - 1))
            nc.vector.tensor_scalar_add(
                qt_sb[:, mt, ns], ps[:, 0, :], bq_sb[:, mt:mt + 1])

        # Q slab 0 upfront (attention on qt=0 needs it)
        for mt in range(4):
            q_proj_tile(0, mt)

        # ---------------- output projection helper ---------------------------
        def outproj_tile(tt, dn, ob_box):
            ts_ = slice(tt * 128, (tt + 1) * 128)
            ds_ = slice(dn * 512, (dn + 1) * 512)
            if dn == 0:
                ob_box[0] = op.tile([128, D], F16, tag="ob", name=f"ob{tt}")
            ps = sc_tile(f"op{tt}{dn}")
            for c in range(4):
                nc.tensor.matmul(
                    ps[:, 0, :], lhsT=ctxT_sb[:, c, ts_], rhs=wo_sb[:, c, ds_],
                    start=(c == 0), stop=(c == 3))
            nc.vector.tensor_copy(ob_box[0][:, ds_], ps[:, 0, :])
            if dn == 3:
                nc.sync.dma_start(out=out_h[ts_, :], in_=ob_box[0])

        # ---------------- background PE work queue ---------------------------
        # Items: (cost_ns, label, emit_fn). Drained between attention groups
        # to keep the PE busy during exp-latency bubbles. PE executes in
        # program order, so any producer a later instruction waits on must
        # already be emitted - hence the forced per-slab drain below.
        bg_queue = deque()
        budget = [0.0]

        def bg_tick(credit):
            budget[0] += credit
            while bg_queue and budget[0] >= bg_queue[0][0]:
                cost, _, fn = bg_queue.popleft()
                fn()
                budget[0] -= cost

        def bg_drain_label(lbl):
            while any(item[1] == lbl for item in bg_queue):
                _, _, fn = bg_queue.popleft()
                fn()
            budget[0] = 0.0

        def bg_drain_all():
            while bg_queue:
                _, _, fn = bg_queue.popleft()
                fn()

        for nt in range(1, 4):
            for mt in range(4):
                bg_queue.append(
                    (3600.0, f"q{nt}",
                     (lambda nt=nt, mt=mt: q_proj_tile(nt, mt))))

        def queue_outproj(qt):
            for tt in range(qt * 4, qt * 4 + 4):
                ob_box = [None]
                for dn in range(4):
                    bg_queue.append(
                        (900.0, f"o{qt}",
                         (lambda tt=tt, dn=dn, ob_box=ob_box:
                          outproj_tile(tt, dn, ob_box))))

        # ---------------- attention main loop -------------------------------
        # Pair mt = heads (mt, mt+4): head strip i uses kv head i.
        for qt in range(NQT):
            if qt > 0:
                bg_drain_label(f"q{qt}")
            qs = slice(qt * 512, (qt + 1) * 512)
            for pair in range(4):
                ctx_ps = [cxps.tile([128, 512], F32, tag="cx", name=f"ctx{i}")
                          for i in range(2)]
                for g in range(NTT):  # one 128-key tile per group
                    # sp slot i = head strip i; one activation covers both
                    sp = sc_tile("sp")
                    ks = slice(g * 128, (g + 1) * 128)
                    for i in range(2):  # head strip i (kv head i)
                        nc.tensor.matmul(
                            sp[:, i, :],
                            lhsT=kt_sb[i * 64:(i + 1) * 64, ks],
                            rhs=qt_sb[i * 64:(i + 1) * 64, pair, qs],
                            start=True, stop=True,
                            tile_position=(i * 64, 0),
                        )
                    e = ep.tile([128, EG, 512], F16)
                    nc.scalar.activation(
                        e[:, :, :], sp[:, :, :],
                        mybir.ActivationFunctionType.Exp, scale=0.125)
                    for i in range(2):
                        nc.tensor.matmul(
                            ctx_ps[i][:, :],
                            lhsT=vaug_sb[:, g, i, :],
                            rhs=e[:, i, :],
                            start=(g == 0), stop=(g == NTT - 1),
                        )
                    if g % 2 == 1:
                        bg_tick(700.0)
                for i in range(2):
                    # custom-DVE ops don't honor partition offsets on HW:
                    # stage the denominator rows into an aligned tile first.
                    den = rp.tile([64, 512], F32, tag="den")
                    rdb = rp.tile([64, 512], F32, tag="rdb")
                    nc.vector.tensor_copy(den, ctx_ps[i][64:128, :])
                    nc.vector.reciprocal_approx_fast(rdb, den)
                    nc.vector.tensor_tensor(
                        ctxT_sb[i * 64:(i + 1) * 64, pair, qs],
                        ctx_ps[i][0:64, :],
                        rdb[:, :],
                        mybir.AluOpType.mult,
                    )
                bg_tick(1500.0)
            queue_outproj(qt)

        # tail: drain all remaining background work (late outproj tiles)
        bg_drain_all()


def _get_nc():
    if "nc" not in _CACHE:
        _CACHE["nc"] = _build_nc()
    return _CACHE["nc"]


# head pairing (h, h+4): local query-dim permutation of the 512 per-core
# Q columns (and matching Wo rows / bq entries)
_PERM = np.concatenate(
    [np.r_[mt * 64:(mt + 1) * 64, (mt + 4) * 64:(mt + 5) * 64] for mt in range(4)])


def kernel(x, Wq, bq, Wk, bk, Wv, bv, Wo, bo, _trace=False):
    x = np.asarray(x, np.float32)
    Wq = np.asarray(Wq, np.float32)
    bq = np.asarray(bq, np.float32)
    Wk = np.asarray(Wk, np.float32)
    bk = np.asarray(bk, np.float32)
    Wv = np.asarray(Wv, np.float32)
    bv = np.asarray(bv, np.float32)
    Wo = np.asarray(Wo, np.float32)
    bo = np.asarray(bo, np.float32)

    nc = _get_nc()
    in_maps = []
    for r in range(8):
        b, g = divmod(r, 4)
        qsl = slice(g * 512, (g + 1) * 512)
        ksl = slice(g * 128, (g + 1) * 128)
        wq_c = Wq[:, qsl][:, _PERM]
        wo_c = Wo[qsl, :][_PERM, :]
        bq_c = bq[qsl][_PERM]
        in_maps.append({
            "xT": np.ascontiguousarray(x[b].T.astype(np.float16)),
            "wq": np.ascontiguousarray(wq_c.astype(np.float16)),
            "wk": np.ascontiguousarray(Wk[:, ksl].astype(np.float16)),
            "wv": np.ascontiguousarray(Wv[:, ksl].astype(np.float16)),
            "wo": np.ascontiguousarray(wo_c.astype(np.float16)),
            "bq2": np.ascontiguousarray(bq_c.reshape(4, 128).T),
            "bk2": np.ascontiguousarray(bk[ksl].reshape(128, 1)),
            "bv2": np.ascontiguousarray(bv[ksl].reshape(128, 1)),
        })

    res = run_bass_kernel_spmd(nc, in_maps, list(range(8)), trace=_trace)
    out = np.zeros((2, S, D), np.float64)
    for r in range(8):
        out[r // 4] += res.results[r]["out"].astype(np.float64)
    out += bo.astype(np.float64)
    result = out.astype(np.float32)
    if _trace:
        return result, res
    return result
